# revision 8
# baseline (speedup 1.0000x reference)
"""Trainium2 Bass kernel for: embedding lookup -> tanh RNN (512 steps) -> dense head.

  tokens [128, 512] int32, V [50000, 256] f32, W [768, 512] f32,
  b [512] f32, Wd [512, 1] f32, bd [1] f32  ->  y [128] f32

Sharding: data-parallel over batch; each of the 8 cores handles 16 rows.
Scan runs in bf16 (fp32 PSUM accumulation); verified rel-err ~4e-3.
"""
import os
import numpy as np
import ml_dtypes
from contextlib import ExitStack

import concourse.bass as bass
import concourse.tile as tile
import concourse.mybir as mybir
from concourse import bacc
from concourse.bass_utils import run_bass_kernel_spmd

BF16 = ml_dtypes.bfloat16
F32 = mybir.dt.float32
BF = mybir.dt.bfloat16
I32 = mybir.dt.int32

P = 128
VOCAB, EMB, HID = 50000, 256, 512
BATCH, SEQ = 128, 512
NCORES = 8
BLOC = BATCH // NCORES            # 16 rows per core
NTOK = BLOC * SEQ                 # 8192 tokens per core
NGT = NTOK // P                   # 64 gather tiles
GT_PER_CH = 4                     # gather tiles per chunk
CH = NGT // GT_PER_CH             # 16 chunks of 512 tokens
CHTOK = P * GT_PER_CH             # 512 tokens per chunk
KT = HID // P                     # 4 k-tiles over hidden
MT = HID // P                     # 4 m-tiles over hidden
KE = EMB // P                     # 2 k-tiles over embedding
NSTEPS = int(os.environ.get("RNN_NSTEPS", SEQ))


def build():
    nc = bacc.Bacc("TRN2", target_bir_lowering=False, debug=False)

    V = nc.dram_tensor("V", [VOCAB, EMB], F32, kind="ExternalInput")
    idxT = nc.dram_tensor("idxT", [P, NGT], I32, kind="ExternalInput")
    Wx_r = nc.dram_tensor("Wx_r", [P, KE * HID], BF, kind="ExternalInput")
    Wh_r = nc.dram_tensor("Wh_r", [P, KT * HID], BF, kind="ExternalInput")
    bvec = nc.dram_tensor("bvec", [P, MT], F32, kind="ExternalInput")
    Wd_r = nc.dram_tensor("Wd_r", [P, MT], BF, kind="ExternalInput")
    bd_t = nc.dram_tensor("bd_t", [1, 1], F32, kind="ExternalInput")
    ident = nc.dram_tensor("ident", [P, P], F32, kind="ExternalInput")
    y_out = nc.dram_tensor("y", [1, BLOC], F32, kind="ExternalOutput")

    with tile.TileContext(nc) as tc, ExitStack() as ctx:
        const = ctx.enter_context(tc.tile_pool(name="const", bufs=1))
        big = ctx.enter_context(tc.tile_pool(name="big", bufs=1))
        gat = ctx.enter_context(tc.tile_pool(name="gat", bufs=4))
        xtp = ctx.enter_context(tc.tile_pool(name="xtp", bufs=3))
        zb = ctx.enter_context(tc.tile_pool(name="zb", bufs=4))

        # ---- constants ----
        ident_sb = const.tile([P, P], F32)
        nc.sync.dma_start(ident_sb[:], ident[:])
        idx_sb = const.tile([P, NGT], I32)
        nc.sync.dma_start(idx_sb[:], idxT[:])
        wx_sb = const.tile([P, KE * HID], BF)
        nc.sync.dma_start(wx_sb[:], Wx_r[:])
        wh_sb = const.tile([P, KT * HID], BF)
        nc.sync.dma_start(wh_sb[:], Wh_r[:])
        bv_sb = const.tile([P, MT], F32)
        nc.sync.dma_start(bv_sb[:], bvec[:])
        wd_sb = const.tile([P, MT], BF)
        nc.sync.dma_start(wd_sb[:], Wd_r[:])
        bd_sb = const.tile([1, 1], F32)
        nc.sync.dma_start(bd_sb[:], bd_t[:])

        # xpT: time-interleaved input projections, col = t*BLOC*MT ... layout
        # [P, SEQ * MT * BLOC] where col = ((t * MT) + m) * BLOC + b_local
        xpT = big.tile([P, SEQ * MT * BLOC], F32)
        # view: [P, t, m, b]
        xpT_v = xpT[:].rearrange("p (t m b) -> p t m b", t=SEQ, m=MT, b=BLOC)

        # ---- phase 1: gather + transpose + input projection ----
        ph1 = ExitStack()
        ps_t = ph1.enter_context(tc.tile_pool(name="ps_t", bufs=2, space="PSUM"))
        ps_xp = ph1.enter_context(tc.tile_pool(name="ps_xp", bufs=2, space="PSUM"))
        for ch in range(CH):
            xT = []  # bf16 [P, CHTOK] per emb k-tile
            xt_tiles = [xtp.tile([P, CHTOK], BF, tag=f"xT{k}", name=f"xT{k}_{ch}")
                        for k in range(KE)]
            for gt in range(GT_PER_CH):
                g = ch * GT_PER_CH + gt
                xg = gat.tile([P, EMB], F32)
                nc.gpsimd.indirect_dma_start(
                    out=xg[:],
                    out_offset=None,
                    in_=V[:],
                    in_offset=bass.IndirectOffsetOnAxis(ap=idx_sb[:, g:g + 1], axis=0),
                )
                for k in range(KE):
                    tp = ps_t.tile([P, P], F32)
                    nc.tensor.transpose(out=tp[:], in_=xg[:, k * P:(k + 1) * P],
                                        identity=ident_sb[:])
                    nc.vector.tensor_copy(xt_tiles[k][:, gt * P:(gt + 1) * P], tp[:])
            # xp matmuls for this chunk: out m-tile = sum_k Wx[k,m].T @ xT[k]
            for m in range(MT):
                pxp = ps_xp.tile([P, CHTOK], F32)
                for k in range(KE):
                    nc.tensor.matmul(
                        pxp[:], wx_sb[:, k * HID + m * P: k * HID + (m + 1) * P],
                        xt_tiles[k][:], start=(k == 0), stop=(k == KE - 1))
                # evacuate with per-partition bias, scattered into the
                # time-interleaved xpT layout: rows t = ch*32 .. ch*32+32
                t0 = ch * (CHTOK // BLOC)
                nc.scalar.activation(
                    xpT_v[:, t0:t0 + CHTOK // BLOC, m, :],
                    pxp[:].rearrange("p (t b) -> p t b", t=CHTOK // BLOC, b=BLOC),
                    mybir.ActivationFunctionType.Identity,
                    bias=bv_sb[:, m:m + 1])

        # ---- phase 2: the scan ----
        ph1.close()
        ps_z = ctx.enter_context(tc.tile_pool(name="ps_z", bufs=3, space="PSUM"))
        ps_y = ctx.enter_context(tc.tile_pool(name="ps_y", bufs=1, space="PSUM"))
        h0 = big.tile([P, KT * BLOC], BF)
        h1 = big.tile([P, KT * BLOC], BF)
        nc.vector.memset(h0[:], 0.0)
        hs = [h0, h1]
        for t in range(NSTEPS):
            cur = hs[t % 2]
            nxt = hs[(t + 1) % 2]
            pz = [ps_z.tile([P, 2 * BLOC], F32, tag=f"pz{i}", name=f"pz{i}_{t}")
                  for i in range(2)]
            # k-phase A: k in {0,1} for all m, then k-phase B: k in {2,3}.
            # Phase A only needs the first half of cur (written by tanh_half0
            # of the previous step), letting PE start before tanh_half1 lands.
            for ph in range(2):
                for m in range(MT):
                    half, mloc = divmod(m, 2)
                    for kk in range(2):
                        k = ph * 2 + kk
                        # start=True clears has_written for the WHOLE bank, so
                        # only the first matmul into each psum tile may set it;
                        # the second m-group's first write lands on cleared
                        # bits and overwrites (per-element semantics).
                        nc.tensor.matmul(
                            pz[half][:, mloc * BLOC:(mloc + 1) * BLOC],
                            wh_sb[:, k * HID + m * P: k * HID + (m + 1) * P],
                            cur[:, k * BLOC:(k + 1) * BLOC],
                            start=(mloc == 0 and k == 0),
                            stop=(mloc == 1 and k == KT - 1),
                            skip_group_check=True)
            for half in range(2):
                zf = zb.tile([P, 2 * BLOC], F32)
                nc.vector.tensor_add(zf[:], pz[half][:],
                                     xpT[:, (t * MT + half * 2) * BLOC:
                                            (t * MT + half * 2 + 2) * BLOC])
                nc.scalar.activation(nxt[:, half * 2 * BLOC:(half + 1) * 2 * BLOC],
                                     zf[:], mybir.ActivationFunctionType.Tanh)

        # ---- phase 3: head ----
        hf = hs[NSTEPS % 2]
        py = ps_y.tile([1, BLOC], F32, tag="py")
        for m in range(MT):
            nc.tensor.matmul(py[:], wd_sb[:, m:m + 1],
                             hf[:, m * BLOC:(m + 1) * BLOC],
                             start=(m == 0), stop=(m == MT - 1))
        y_sb = zb.tile([1, BLOC], F32, tag="ysb")
        nc.scalar.activation(y_sb[:], py[:],
                             mybir.ActivationFunctionType.Identity,
                             bias=bd_sb[:, :1])
        nc.sync.dma_start(y_out[:], y_sb[:])

    nc.compile()
    return nc


_CACHED = None


def _get_nc():
    global _CACHED
    if _CACHED is None:
        _CACHED = build()
    return _CACHED


def _prep_inputs(tokens, V, W, b, Wd, bd):
    tokens = np.asarray(tokens, dtype=np.int32)
    V = np.ascontiguousarray(np.asarray(V, dtype=np.float32))
    W = np.asarray(W, dtype=np.float32)
    b = np.asarray(b, dtype=np.float32)
    Wd = np.asarray(Wd, dtype=np.float32)
    bd = np.asarray(bd, dtype=np.float32)

    Wx, Wh = W[:EMB], W[EMB:]
    Wx_r = np.concatenate([Wx[k * P:(k + 1) * P] for k in range(KE)],
                          axis=1).astype(BF16)          # [P, KE*HID]
    Wh_r = np.concatenate([Wh[k * P:(k + 1) * P] for k in range(KT)],
                          axis=1).astype(BF16)          # [P, KT*HID]
    bvec = np.ascontiguousarray(b.reshape(MT, P).T, dtype=np.float32)
    Wd_r = np.ascontiguousarray(Wd[:, 0].reshape(MT, P).T).astype(BF16)
    bd_t = np.array([[bd.reshape(-1)[0]]], dtype=np.float32)
    identm = np.eye(P, dtype=np.float32)

    in_maps = []
    for c in range(NCORES):
        tc_ = tokens[c * BLOC:(c + 1) * BLOC]           # [BLOC, SEQ]
        flat = tc_.T.reshape(-1)                        # j = t*BLOC + b
        idxT = np.ascontiguousarray(flat.reshape(NGT, P).T, dtype=np.int32)
        in_maps.append({
            "V": V, "idxT": idxT, "Wx_r": Wx_r, "Wh_r": Wh_r,
            "bvec": bvec, "Wd_r": Wd_r, "bd_t": bd_t, "ident": identm,
        })
    return in_maps


def kernel(tokens, V, W, b, Wd, bd):
    nc = _get_nc()
    in_maps = _prep_inputs(tokens, V, W, b, Wd, bd)
    res = run_bass_kernel_spmd(nc, in_maps, core_ids=list(range(NCORES)))
    y = np.concatenate([res.results[c]["y"].reshape(-1) for c in range(NCORES)])
    return y.astype(np.float32)


# revision 11
# speedup vs baseline: 1.1855x; 1.1855x over previous
"""Trainium2 Bass kernel for: embedding lookup -> tanh RNN (512 steps) -> dense head.

  tokens [128, 512] int32, V [50000, 256] f32, W [768, 512] f32,
  b [512] f32, Wd [512, 1] f32, bd [1] f32  ->  y [128] f32

Sharding: data-parallel over batch; each of the 8 cores handles 16 rows.
Scan runs in bf16 (fp32 PSUM accumulation); verified rel-err ~4e-3.
"""
import os
import numpy as np
import ml_dtypes
from contextlib import ExitStack

import concourse.bass as bass
import concourse.tile as tile
import concourse.mybir as mybir
from concourse import bacc
from concourse.bass_utils import run_bass_kernel_spmd

BF16 = ml_dtypes.bfloat16
F32 = mybir.dt.float32
BF = mybir.dt.bfloat16
I32 = mybir.dt.int32

P = 128
VOCAB, EMB, HID = 50000, 256, 512
BATCH, SEQ = 128, 512
NCORES = 8
BLOC = BATCH // NCORES            # 16 rows per core
NTOK = BLOC * SEQ                 # 8192 tokens per core
NGT = NTOK // P                   # 64 gather tiles
GT_PER_CH = 4                     # gather tiles per chunk
CH = NGT // GT_PER_CH             # 16 chunks of 512 tokens
CHTOK = P * GT_PER_CH             # 512 tokens per chunk
KT = HID // P                     # 4 k-tiles over hidden
MT = HID // P                     # 4 m-tiles over hidden
KE = EMB // P                     # 2 k-tiles over embedding
NSTEPS = int(os.environ.get("RNN_NSTEPS", SEQ))


def build():
    nc = bacc.Bacc("TRN2", target_bir_lowering=False, debug=False)

    V = nc.dram_tensor("V", [VOCAB, EMB], F32, kind="ExternalInput")
    idxT = nc.dram_tensor("idxT", [P, NGT], I32, kind="ExternalInput")
    Wx_r = nc.dram_tensor("Wx_r", [P, KE * HID], BF, kind="ExternalInput")
    Wh_r = nc.dram_tensor("Wh_r", [P, KT * HID], BF, kind="ExternalInput")
    bvec = nc.dram_tensor("bvec", [P, MT], F32, kind="ExternalInput")
    Wd_r = nc.dram_tensor("Wd_r", [P, MT], BF, kind="ExternalInput")
    bd_t = nc.dram_tensor("bd_t", [1, 1], F32, kind="ExternalInput")
    ident = nc.dram_tensor("ident", [P, P], F32, kind="ExternalInput")
    y_out = nc.dram_tensor("y", [1, BLOC], F32, kind="ExternalOutput")

    with tile.TileContext(nc) as tc, ExitStack() as ctx:
        const = ctx.enter_context(tc.tile_pool(name="const", bufs=1))
        big = ctx.enter_context(tc.tile_pool(name="big", bufs=1))
        gat = ctx.enter_context(tc.tile_pool(name="gat", bufs=4))
        xtp = ctx.enter_context(tc.tile_pool(name="xtp", bufs=3))
        zb = ctx.enter_context(tc.tile_pool(name="zb", bufs=4))

        # ---- constants ----
        ident_sb = const.tile([P, P], F32)
        nc.sync.dma_start(ident_sb[:], ident[:])
        idx_sb = const.tile([P, NGT], I32)
        nc.sync.dma_start(idx_sb[:], idxT[:])
        wx_sb = const.tile([P, KE * HID], BF)
        nc.sync.dma_start(wx_sb[:], Wx_r[:])
        wh_sb = const.tile([P, KT * HID], BF)
        nc.sync.dma_start(wh_sb[:], Wh_r[:])
        bv_sb = const.tile([P, MT], F32)
        nc.sync.dma_start(bv_sb[:], bvec[:])
        wd_sb = const.tile([P, MT], BF)
        nc.sync.dma_start(wd_sb[:], Wd_r[:])
        bd_sb = const.tile([1, 1], F32)
        nc.sync.dma_start(bd_sb[:], bd_t[:])

        # xpT: time-interleaved input projections, col = t*BLOC*MT ... layout
        # [P, SEQ * MT * BLOC] where col = ((t * MT) + m) * BLOC + b_local
        xpT = big.tile([P, SEQ * MT * BLOC], BF)
        # view: [P, t, m, b]
        xpT_v = xpT[:].rearrange("p (t m b) -> p t m b", t=SEQ, m=MT, b=BLOC)

        # ---- phase 1: gather + transpose + input projection ----
        ph1 = ExitStack()
        ps_t = ph1.enter_context(tc.tile_pool(name="ps_t", bufs=2, space="PSUM"))
        ps_xp = ph1.enter_context(tc.tile_pool(name="ps_xp", bufs=2, space="PSUM"))
        for ch in range(CH):
            xT = []  # bf16 [P, CHTOK] per emb k-tile
            xt_tiles = [xtp.tile([P, CHTOK], BF, tag=f"xT{k}", name=f"xT{k}_{ch}")
                        for k in range(KE)]
            for gt in range(GT_PER_CH):
                g = ch * GT_PER_CH + gt
                xg = gat.tile([P, EMB], F32)
                nc.gpsimd.indirect_dma_start(
                    out=xg[:],
                    out_offset=None,
                    in_=V[:],
                    in_offset=bass.IndirectOffsetOnAxis(ap=idx_sb[:, g:g + 1], axis=0),
                )
                for k in range(KE):
                    tp = ps_t.tile([P, P], F32)
                    nc.tensor.transpose(out=tp[:], in_=xg[:, k * P:(k + 1) * P],
                                        identity=ident_sb[:])
                    nc.vector.tensor_copy(xt_tiles[k][:, gt * P:(gt + 1) * P], tp[:])
            # xp matmuls for this chunk: out m-tile = sum_k Wx[k,m].T @ xT[k]
            for m in range(MT):
                pxp = ps_xp.tile([P, CHTOK], F32)
                for k in range(KE):
                    nc.tensor.matmul(
                        pxp[:], wx_sb[:, k * HID + m * P: k * HID + (m + 1) * P],
                        xt_tiles[k][:], start=(k == 0), stop=(k == KE - 1))
                # evacuate with per-partition bias, scattered into the
                # time-interleaved xpT layout: rows t = ch*32 .. ch*32+32
                t0 = ch * (CHTOK // BLOC)
                nc.vector.tensor_scalar_add(
                    xpT_v[:, t0:t0 + CHTOK // BLOC, m, :],
                    pxp[:].rearrange("p (t b) -> p t b", t=CHTOK // BLOC, b=BLOC),
                    bv_sb[:, m:m + 1])

        # ---- phase 2: the scan ----
        ph1.close()
        ps_z = ctx.enter_context(tc.tile_pool(name="ps_z", bufs=3, space="PSUM"))
        ps_y = ctx.enter_context(tc.tile_pool(name="ps_y", bufs=1, space="PSUM"))
        h0 = big.tile([P, KT * BLOC], BF)
        h1 = big.tile([P, KT * BLOC], BF)
        nc.vector.memset(h0[:], 0.0)
        hs = [h0, h1]
        id_bf = const.tile([P, P], BF)
        nc.vector.tensor_copy(id_bf[:], ident_sb[:])
        for t in range(NSTEPS):
            cur = hs[t % 2]
            nxt = hs[(t + 1) % 2]
            pz = [ps_z.tile([P, 2 * BLOC], F32, tag=f"pz{i}", name=f"pz{i}_{t}")
                  for i in range(2)]
            # Per bank h (holding m = 2h, 2h+1): start MM, xp-inject via
            # identity matmul (lands on cleared has_written bits for the
            # second m-group -> overwrite; accumulate for the first), then
            # the remaining MMs ordered k01 before k23 so the first half of
            # the bank block only needs tanh_half0 of the previous step.
            for half in range(2):
                ma, mb = 2 * half, 2 * half + 1
                nc.tensor.matmul(
                    pz[half][:, 0:BLOC],
                    wh_sb[:, 0 * HID + ma * P: 0 * HID + (ma + 1) * P],
                    cur[:, 0:BLOC], start=True, stop=False,
                    skip_group_check=True)
                nc.tensor.matmul(
                    pz[half][:], id_bf[:],
                    xpT[:, (t * MT + 2 * half) * BLOC:
                           (t * MT + 2 * half + 2) * BLOC],
                    start=False, stop=False, skip_group_check=True)
                for m, ks in ((ma, (1,)), (mb, (0, 1)), (ma, (2, 3)),
                              (mb, (2, 3))):
                    mloc = m - 2 * half
                    for k in ks:
                        nc.tensor.matmul(
                            pz[half][:, mloc * BLOC:(mloc + 1) * BLOC],
                            wh_sb[:, k * HID + m * P: k * HID + (m + 1) * P],
                            cur[:, k * BLOC:(k + 1) * BLOC],
                            start=False,
                            stop=(mloc == 1 and k == KT - 1),
                            skip_group_check=True)
                nc.scalar.activation(nxt[:, half * 2 * BLOC:(half + 1) * 2 * BLOC],
                                     pz[half][:],
                                     mybir.ActivationFunctionType.Tanh)

        # ---- phase 3: head ----
        hf = hs[NSTEPS % 2]
        py = ps_y.tile([1, BLOC], F32, tag="py")
        for m in range(MT):
            nc.tensor.matmul(py[:], wd_sb[:, m:m + 1],
                             hf[:, m * BLOC:(m + 1) * BLOC],
                             start=(m == 0), stop=(m == MT - 1))
        y_sb = zb.tile([1, BLOC], F32, tag="ysb")
        nc.scalar.activation(y_sb[:], py[:],
                             mybir.ActivationFunctionType.Identity,
                             bias=bd_sb[:, :1])
        nc.sync.dma_start(y_out[:], y_sb[:])

    nc.compile()
    return nc


_CACHED = None


def _get_nc():
    global _CACHED
    if _CACHED is None:
        _CACHED = build()
    return _CACHED


def _prep_inputs(tokens, V, W, b, Wd, bd):
    tokens = np.asarray(tokens, dtype=np.int32)
    V = np.ascontiguousarray(np.asarray(V, dtype=np.float32))
    W = np.asarray(W, dtype=np.float32)
    b = np.asarray(b, dtype=np.float32)
    Wd = np.asarray(Wd, dtype=np.float32)
    bd = np.asarray(bd, dtype=np.float32)

    Wx, Wh = W[:EMB], W[EMB:]
    Wx_r = np.concatenate([Wx[k * P:(k + 1) * P] for k in range(KE)],
                          axis=1).astype(BF16)          # [P, KE*HID]
    Wh_r = np.concatenate([Wh[k * P:(k + 1) * P] for k in range(KT)],
                          axis=1).astype(BF16)          # [P, KT*HID]
    bvec = np.ascontiguousarray(b.reshape(MT, P).T, dtype=np.float32)
    Wd_r = np.ascontiguousarray(Wd[:, 0].reshape(MT, P).T).astype(BF16)
    bd_t = np.array([[bd.reshape(-1)[0]]], dtype=np.float32)
    identm = np.eye(P, dtype=np.float32)

    in_maps = []
    for c in range(NCORES):
        tc_ = tokens[c * BLOC:(c + 1) * BLOC]           # [BLOC, SEQ]
        flat = tc_.T.reshape(-1)                        # j = t*BLOC + b
        idxT = np.ascontiguousarray(flat.reshape(NGT, P).T, dtype=np.int32)
        in_maps.append({
            "V": V, "idxT": idxT, "Wx_r": Wx_r, "Wh_r": Wh_r,
            "bvec": bvec, "Wd_r": Wd_r, "bd_t": bd_t, "ident": identm,
        })
    return in_maps


def kernel(tokens, V, W, b, Wd, bd):
    nc = _get_nc()
    in_maps = _prep_inputs(tokens, V, W, b, Wd, bd)
    res = run_bass_kernel_spmd(nc, in_maps, core_ids=list(range(NCORES)))
    y = np.concatenate([res.results[c]["y"].reshape(-1) for c in range(NCORES)])
    return y.astype(np.float32)


# revision 12
# speedup vs baseline: 1.2619x; 1.0644x over previous
"""Trainium2 Bass kernel for: embedding lookup -> tanh RNN (512 steps) -> dense head.

  tokens [128, 512] int32, V [50000, 256] f32, W [768, 512] f32,
  b [512] f32, Wd [512, 1] f32, bd [1] f32  ->  y [128] f32

Sharding: data-parallel over batch; each of the 8 cores handles 16 rows.
Scan runs in bf16 (fp32 PSUM accumulation); verified rel-err ~4e-3.
"""
import os
import numpy as np
import ml_dtypes
from contextlib import ExitStack

import concourse.bass as bass
import concourse.tile as tile
import concourse.mybir as mybir
from concourse import bacc
from concourse.bass_utils import run_bass_kernel_spmd

BF16 = ml_dtypes.bfloat16
F32 = mybir.dt.float32
BF = mybir.dt.bfloat16
I32 = mybir.dt.int32

P = 128
VOCAB, EMB, HID = 50000, 256, 512
BATCH, SEQ = 128, 512
NCORES = 8
BLOC = BATCH // NCORES            # 16 rows per core
NTOK = BLOC * SEQ                 # 8192 tokens per core
NGT = NTOK // P                   # 64 gather tiles
GT_PER_CH = 4                     # gather tiles per chunk
CH = NGT // GT_PER_CH             # 16 chunks of 512 tokens
CHTOK = P * GT_PER_CH             # 512 tokens per chunk
KT = HID // P                     # 4 k-tiles over hidden
MT = HID // P                     # 4 m-tiles over hidden
KE = EMB // P                     # 2 k-tiles over embedding
NSTEPS = int(os.environ.get("RNN_NSTEPS", SEQ))


def build():
    nc = bacc.Bacc("TRN2", target_bir_lowering=False, debug=False)

    V = nc.dram_tensor("V", [VOCAB, EMB], F32, kind="ExternalInput")
    idxT = nc.dram_tensor("idxT", [P, NGT], I32, kind="ExternalInput")
    Wx_r = nc.dram_tensor("Wx_r", [P, KE * HID], BF, kind="ExternalInput")
    Wh_r = nc.dram_tensor("Wh_r", [P, KT * HID], BF, kind="ExternalInput")
    bvec = nc.dram_tensor("bvec", [P, MT], F32, kind="ExternalInput")
    Wd_r = nc.dram_tensor("Wd_r", [P, MT], BF, kind="ExternalInput")
    bd_t = nc.dram_tensor("bd_t", [1, 1], F32, kind="ExternalInput")
    ident = nc.dram_tensor("ident", [P, P], F32, kind="ExternalInput")
    y_out = nc.dram_tensor("y", [1, BLOC], F32, kind="ExternalOutput")

    with tile.TileContext(nc) as tc, ExitStack() as ctx:
        const = ctx.enter_context(tc.tile_pool(name="const", bufs=1))
        big = ctx.enter_context(tc.tile_pool(name="big", bufs=1))
        gat = ctx.enter_context(tc.tile_pool(name="gat", bufs=4))
        xtp = ctx.enter_context(tc.tile_pool(name="xtp", bufs=3))
        zb = ctx.enter_context(tc.tile_pool(name="zb", bufs=4))

        # ---- constants ----
        ident_sb = const.tile([P, P], F32)
        nc.sync.dma_start(ident_sb[:], ident[:])
        idx_sb = const.tile([P, NGT], I32)
        nc.sync.dma_start(idx_sb[:], idxT[:])
        wx_sb = const.tile([P, KE * HID], BF)
        nc.sync.dma_start(wx_sb[:], Wx_r[:])
        wh_sb = const.tile([P, KT * HID], BF)
        nc.sync.dma_start(wh_sb[:], Wh_r[:])
        bv_sb = const.tile([P, MT], F32)
        nc.sync.dma_start(bv_sb[:], bvec[:])
        wd_sb = const.tile([P, MT], BF)
        nc.sync.dma_start(wd_sb[:], Wd_r[:])
        bd_sb = const.tile([1, 1], F32)
        nc.sync.dma_start(bd_sb[:], bd_t[:])

        # xpT: time-interleaved input projections, col = t*BLOC*MT ... layout
        # [P, SEQ * MT * BLOC] where col = ((t * MT) + m) * BLOC + b_local
        xpT = big.tile([P, SEQ * MT * BLOC], BF)
        # view: [P, t, m, b]
        xpT_v = xpT[:].rearrange("p (t m b) -> p t m b", t=SEQ, m=MT, b=BLOC)

        # ---- phase 1: gather + transpose + input projection ----
        ph1 = ExitStack()
        ps_t = ph1.enter_context(tc.tile_pool(name="ps_t", bufs=2, space="PSUM"))
        ps_xp = ph1.enter_context(tc.tile_pool(name="ps_xp", bufs=2, space="PSUM"))
        for ch in range(CH):
            xT = []  # bf16 [P, CHTOK] per emb k-tile
            xt_tiles = [xtp.tile([P, CHTOK], BF, tag=f"xT{k}", name=f"xT{k}_{ch}")
                        for k in range(KE)]
            for gt in range(GT_PER_CH):
                g = ch * GT_PER_CH + gt
                xg = gat.tile([P, EMB], F32)
                nc.gpsimd.indirect_dma_start(
                    out=xg[:],
                    out_offset=None,
                    in_=V[:],
                    in_offset=bass.IndirectOffsetOnAxis(ap=idx_sb[:, g:g + 1], axis=0),
                )
                for k in range(KE):
                    tp = ps_t.tile([P, P], F32)
                    nc.tensor.transpose(out=tp[:], in_=xg[:, k * P:(k + 1) * P],
                                        identity=ident_sb[:])
                    nc.vector.tensor_copy(xt_tiles[k][:, gt * P:(gt + 1) * P], tp[:])
            # xp matmuls for this chunk: out m-tile = sum_k Wx[k,m].T @ xT[k]
            for m in range(MT):
                pxp = ps_xp.tile([P, CHTOK], F32)
                for k in range(KE):
                    nc.tensor.matmul(
                        pxp[:], wx_sb[:, k * HID + m * P: k * HID + (m + 1) * P],
                        xt_tiles[k][:], start=(k == 0), stop=(k == KE - 1))
                # evacuate with per-partition bias, scattered into the
                # time-interleaved xpT layout: rows t = ch*32 .. ch*32+32
                t0 = ch * (CHTOK // BLOC)
                nc.vector.tensor_scalar_add(
                    xpT_v[:, t0:t0 + CHTOK // BLOC, m, :],
                    pxp[:].rearrange("p (t b) -> p t b", t=CHTOK // BLOC, b=BLOC),
                    bv_sb[:, m:m + 1])

        # ---- phase 2: the scan ----
        ph1.close()
        ps_z = ctx.enter_context(tc.tile_pool(name="ps_z", bufs=3, space="PSUM"))
        ps_y = ctx.enter_context(tc.tile_pool(name="ps_y", bufs=1, space="PSUM"))
        h0 = big.tile([P, KT * BLOC], BF)
        h1 = big.tile([P, KT * BLOC], BF)
        nc.vector.memset(h0[:], 0.0)
        hs = [h0, h1]
        id_bf = const.tile([P, P], BF)
        nc.vector.tensor_copy(id_bf[:], ident_sb[:])
        for t in range(NSTEPS):
            cur = hs[t % 2]
            nxt = hs[(t + 1) % 2]
            pz = [ps_z.tile([P, 2 * BLOC], F32, tag=f"pz{i}", name=f"pz{i}_{t}")
                  for i in range(2)]
            # xp-inject via identity matmul opens each bank (start=True clears
            # has_written bank-wide, so both m-groups' weight MMs accumulate
            # on top). The inject depends only on xpT, so the scheduler can
            # run it during the previous step's tanh tail.
            for half in range(2):
                nc.tensor.matmul(
                    pz[half][:], id_bf[:],
                    xpT[:, (t * MT + 2 * half) * BLOC:
                           (t * MT + 2 * half + 2) * BLOC],
                    start=True, stop=False, skip_group_check=True)
            last_w = None
            for half in range(2):
                ma, mb = 2 * half, 2 * half + 1
                order = [(ma, 0), (ma, 1), (mb, 0), (mb, 1),
                         (ma, 2), (ma, 3), (mb, 2), (mb, 3)]
                for i, (m, k) in enumerate(order):
                    mloc = m - 2 * half
                    w = nc.tensor.matmul(
                        pz[half][:, mloc * BLOC:(mloc + 1) * BLOC],
                        wh_sb[:, k * HID + m * P: k * HID + (m + 1) * P],
                        cur[:, k * BLOC:(k + 1) * BLOC],
                        start=False, stop=(i == len(order) - 1),
                        skip_group_check=True)
                    # keep bank0's weight MMs ahead of bank1's so bank0's
                    # group closes early and tanh_half0 starts mid-stream
                    if half == 1 and i == 0 and last_w is not None:
                        tile.add_dep_helper(w.ins, last_w.ins, sync=False,
                                            reason="bank order")
                if half == 0:
                    last_w = w
                nc.scalar.activation(nxt[:, half * 2 * BLOC:(half + 1) * 2 * BLOC],
                                     pz[half][:],
                                     mybir.ActivationFunctionType.Tanh)

        # ---- phase 3: head ----
        hf = hs[NSTEPS % 2]
        py = ps_y.tile([1, BLOC], F32, tag="py")
        for m in range(MT):
            nc.tensor.matmul(py[:], wd_sb[:, m:m + 1],
                             hf[:, m * BLOC:(m + 1) * BLOC],
                             start=(m == 0), stop=(m == MT - 1))
        y_sb = zb.tile([1, BLOC], F32, tag="ysb")
        nc.scalar.activation(y_sb[:], py[:],
                             mybir.ActivationFunctionType.Identity,
                             bias=bd_sb[:, :1])
        nc.sync.dma_start(y_out[:], y_sb[:])

    nc.compile()
    return nc


_CACHED = None


def _get_nc():
    global _CACHED
    if _CACHED is None:
        _CACHED = build()
    return _CACHED


def _prep_inputs(tokens, V, W, b, Wd, bd):
    tokens = np.asarray(tokens, dtype=np.int32)
    V = np.ascontiguousarray(np.asarray(V, dtype=np.float32))
    W = np.asarray(W, dtype=np.float32)
    b = np.asarray(b, dtype=np.float32)
    Wd = np.asarray(Wd, dtype=np.float32)
    bd = np.asarray(bd, dtype=np.float32)

    Wx, Wh = W[:EMB], W[EMB:]
    Wx_r = np.concatenate([Wx[k * P:(k + 1) * P] for k in range(KE)],
                          axis=1).astype(BF16)          # [P, KE*HID]
    Wh_r = np.concatenate([Wh[k * P:(k + 1) * P] for k in range(KT)],
                          axis=1).astype(BF16)          # [P, KT*HID]
    bvec = np.ascontiguousarray(b.reshape(MT, P).T, dtype=np.float32)
    Wd_r = np.ascontiguousarray(Wd[:, 0].reshape(MT, P).T).astype(BF16)
    bd_t = np.array([[bd.reshape(-1)[0]]], dtype=np.float32)
    identm = np.eye(P, dtype=np.float32)

    in_maps = []
    for c in range(NCORES):
        tc_ = tokens[c * BLOC:(c + 1) * BLOC]           # [BLOC, SEQ]
        flat = tc_.T.reshape(-1)                        # j = t*BLOC + b
        idxT = np.ascontiguousarray(flat.reshape(NGT, P).T, dtype=np.int32)
        in_maps.append({
            "V": V, "idxT": idxT, "Wx_r": Wx_r, "Wh_r": Wh_r,
            "bvec": bvec, "Wd_r": Wd_r, "bd_t": bd_t, "ident": identm,
        })
    return in_maps


def kernel(tokens, V, W, b, Wd, bd):
    nc = _get_nc()
    in_maps = _prep_inputs(tokens, V, W, b, Wd, bd)
    res = run_bass_kernel_spmd(nc, in_maps, core_ids=list(range(NCORES)))
    y = np.concatenate([res.results[c]["y"].reshape(-1) for c in range(NCORES)])
    return y.astype(np.float32)
